# revision 3
# baseline (speedup 1.0000x reference)
"""Trainium2 Bass kernel for nn_CombinedLossExp71 (combined distillation loss).

Sharding: data-parallel over B across 8 cores, codebook replicated.
Each core b handles batch row b (1500 tokens x 512 dims):
  - fp16 matmuls (full PE rate) produce raw logit pieces ps = 2*x.c in PSUM
    per (125-token, 2048-code) half; 4 rounds per tile (s0,s1,t0,t1).
  - teacher path: DVE drains PSUM as mnl_t = c2(fp32) - ps (fp16 out), then
    an exact full-row fp16 min-reduce gives nm_t = -max(logits). No
    subsampling -> exp can never overflow.
  - student path: ACT copies PSUM to SBUF fp16; Pool does the c2 sub
    (all-fp16), DVE min-reduce gives nm_s (= exact min distance piece for
    the VQ term).
  - ACT exp(-mnl + nm) with accum_out yields e_t / se_t / se_s in one op
    per feature.
  - dot = sum(e_t*(l_t - l_s)) is split as  sum(e_t*mnl_s) - sum(e_t*mnl_t):
    two DVE scalar_tensor_tensor passes with fused sum accumulation; the
    difference is taken on (125,12) stats in the final combine. This removes
    the full-K delta pass entirely.
  - feature/triplet/x2 via Pool subs + DVE stt square-accum on fp16 naturals.
  Per-core outputs are 4 partial sums [feature, triplet, kl, vq]; the final
  masked-mean combination happens on host (scalar work only).

Self-contained: hardcodes shapes for B=8, T=1500, D=512, K=4096, STRIDE=320.
"""
import numpy as np

try:
    import concourse.bass as bass
except ImportError:  # environment fallback
    import sys

    sys.path.insert(0, "/opt/trn_rl_repo")
    import concourse.bass as bass

import concourse.tile as tile
from concourse import mybir
from concourse.bass_utils import run_bass_kernel_spmd

B, T, D, K = 8, 1500, 512, 4096
STRIDE = 320
P = 125          # tokens per tile (partition dim)
NT = T // P      # 12 tiles
KH = K // 2      # codes per PSUM round
NC = 8           # cores
F32 = mybir.dt.float32
F16 = mybir.dt.float16

Act = mybir.ActivationFunctionType
Alu = mybir.AluOpType
AxX = mybir.AxisListType.X


def _split_sync_waits(nc, max_waits=1):
    """This container's walrus supports only one embedded sync-wait per
    instruction; move excess waits onto inserted same-engine NoOps."""
    counter = [0]
    for f in nc.m.functions:
        for bb in f.blocks:
            insts = bb.instructions
            out = []
            changed = False
            for ins in insts:
                si = ins.sync_info
                waits = list(si.on_wait) if si is not None and si.on_wait else []
                if len(waits) > max_waits:
                    changed = True
                    extra, keep = waits[:-max_waits], waits[-max_waits:]
                    for j in range(0, len(extra), max_waits):
                        counter[0] += 1
                        nop = mybir.InstNoOp(
                            name=f"wsplit-{counter[0]}",
                            ins=[],
                            outs=[],
                            engine=ins.engine,
                        )
                        nop.sync_info = mybir.SyncInfo(
                            on_wait=extra[j : j + max_waits], on_update=[]
                        )
                        nc.register_instruction(nop, overwrite=True)
                        out.append(nop)
                    si.on_wait = keep
                out.append(ins)
            if changed:
                insts.clear()
                insts.extend(out)


def _build(dump=False):
    nc = bass.Bass()

    s_nat = nc.dram_tensor("s_nat", [T, D], F16, kind="ExternalInput")
    t_nat = nc.dram_tensor("t_nat", [T, D], F16, kind="ExternalInput")
    tp_nat = nc.dram_tensor("tp_nat", [T, D], F16, kind="ExternalInput")
    st2_tr = nc.dram_tensor("st2_tr", [D, T], F16, kind="ExternalInput")  # 2*S^T
    tt2_tr = nc.dram_tensor("tt2_tr", [D, T], F16, kind="ExternalInput")  # 2*T^T
    ct_tr = nc.dram_tensor("ct_tr", [D, K], F16, kind="ExternalInput")
    c2b_in = nc.dram_tensor("c2b", [128, K], F32, kind="ExternalInput")
    c2h_in = nc.dram_tensor("c2h", [128, K], F16, kind="ExternalInput")
    mask_in = nc.dram_tensor("maskT", [P, NT], F32, kind="ExternalInput")
    out_d = nc.dram_tensor("partials", [1, 4], F32, kind="ExternalOutput")
    dbg_d = nc.dram_tensor("dbg", [P, 12 * NT], F32, kind="ExternalOutput") if dump else None

    with tile.TileContext(nc) as tc:
        with (
            tc.tile_pool(name="const", bufs=1) as const,
            tc.tile_pool(name="stats", bufs=1) as stats,
            tc.tile_pool(name="w", bufs=16) as wpool,
            tc.tile_pool(name="nat", bufs=2) as natpool,
            tc.tile_pool(name="mnl", bufs=2) as mnlpool,
            tc.tile_pool(name="psc", bufs=2) as pscpool,
            tc.tile_pool(name="eh", bufs=2) as ehpool,
            tc.tile_pool(name="scratch", bufs=1) as scratchpool,
            tc.tile_pool(name="sq", bufs=2) as sqpool,
            tc.tile_pool(name="psum", bufs=2, space="PSUM") as psum,
        ):
            # ---- constants ----
            ct_sb = []
            for d in range(4):
                c = const.tile([128, K], F16, tag=f"ct{d}", name=f"ct{d}")
                nc.sync.dma_start(out=c[:], in_=ct_tr[d * 128 : (d + 1) * 128, :])
                ct_sb.append(c)
            c2b = const.tile([128, K], F32, tag="c2b", name="c2b")
            nc.sync.dma_start(out=c2b[:], in_=c2b_in[:])
            c2h = const.tile([128, K], F16, tag="c2h", name="c2h")
            nc.sync.dma_start(out=c2h[:], in_=c2h_in[:])
            maskT = const.tile([P, NT], F32, tag="mask", name="maskT")
            nc.sync.dma_start(out=maskT[:], in_=mask_in[:])
            ones = const.tile([P, 1], F32, tag="ones", name="ones")
            nc.vector.memset(ones[:], 1.0)
            b_margin = const.tile([P, 1], F32, tag="b_margin", name="b_margin")
            nc.vector.memset(b_margin[:], 0.2)

            # ---- per-token stat arrays (col it per tile) ----
            def stat(name):
                return stats.tile([P, NT], F32, tag=name, name=name)

            nm_t_all = stat("nm_t")      # -max(l_t), exact
            nm_s_all = stat("nm_s")      # -max(l_s), exact
            se_t_all = stat("se_t")
            se_s_all = stat("se_s")
            d1_all = stat("d1")          # sum(e_t * mnl_s)
            d2_all = stat("d2")          # sum(e_t * mnl_t)
            rsq_pos_all = stat("rsq_pos")
            rsq_neg_all = stat("rsq_neg")
            x2_all = stat("x2")

            trans = {"t": tt2_tr, "s": st2_tr}

            for it in range(NT):
                tok = slice(it * P, (it + 1) * P)

                # natural tiles + feature/triplet/x2 pieces
                s_t = natpool.tile([P, D], F16, tag="s_nat", name="s_t")
                t_t = natpool.tile([P, D], F16, tag="t_nat", name="t_t")
                tp_t = natpool.tile([P, D], F16, tag="tp_nat", name="tp_t")
                nc.sync.dma_start(out=s_t[:], in_=s_nat[tok, :])
                nc.sync.dma_start(out=t_t[:], in_=t_nat[tok, :])
                nc.sync.dma_start(out=tp_t[:], in_=tp_nat[tok, :])

                dpos = sqpool.tile([P, D], F16, tag="dpos", name="dpos")
                dneg = sqpool.tile([P, D], F16, tag="dneg", name="dneg")
                nc.gpsimd.tensor_sub(out=dpos[:], in0=s_t[:], in1=t_t[:])
                nc.gpsimd.tensor_sub(out=dneg[:], in0=s_t[:], in1=tp_t[:])
                # squared-sum accums via DVE stt (x*1)*x with fused sum
                sqs = sqpool.tile([P, D], F16, tag="sqscr", name="sqs")
                nc.vector.scalar_tensor_tensor(
                    out=sqs[:], in0=dpos[:], scalar=1.0, in1=dpos[:],
                    op0=Alu.mult, op1=Alu.mult,
                    accum_out=rsq_pos_all[:, it : it + 1],
                )
                sqs2 = sqpool.tile([P, D], F16, tag="sqscr", name="sqs2")
                nc.vector.scalar_tensor_tensor(
                    out=sqs2[:], in0=dneg[:], scalar=1.0, in1=dneg[:],
                    op0=Alu.mult, op1=Alu.mult,
                    accum_out=rsq_neg_all[:, it : it + 1],
                )
                sqs3 = sqpool.tile([P, D], F16, tag="sqscr", name="sqs3")
                nc.vector.scalar_tensor_tensor(
                    out=sqs3[:], in0=s_t[:], scalar=1.0, in1=s_t[:],
                    op0=Alu.mult, op1=Alu.mult,
                    accum_out=x2_all[:, it : it + 1],
                )

                # stationary weight slices (2*S^T, 2*T^T) fp16
                w = {}
                for f in ("t", "s"):
                    w[f] = []
                    for d in range(4):
                        wt = wpool.tile([128, P], F16, tag="w", name=f"w_{f}{d}")
                        nc.sync.dma_start(
                            out=wt[:], in_=trans[f][d * 128 : (d + 1) * 128, tok]
                        )
                        w[f].append(wt)

                mnl_t = mnlpool.tile([P, K], F16, tag="mnl_t", name="mnl_t")
                mnl_s = mnlpool.tile([P, K], F16, tag="mnl_s", name="mnl_s")
                ps16 = pscpool.tile([P, K], F16, tag="ps16", name="ps16")
                e_t = ehpool.tile([P, K], F16, tag="e_t", name="e_t")

                def mm_round(f, half, ps):
                    for d in range(4):
                        for kc in range(4):
                            c0 = half * KH + kc * 512
                            nc.tensor.matmul(
                                out=ps[:, kc * 512 : (kc + 1) * 512],
                                lhsT=w[f][d][:],
                                rhs=ct_sb[d][:, c0 : c0 + 512],
                                start=(d == 0),
                                stop=(d == 3),
                            )

                # student rounds first: PSUM -> fp16 copy on ACT, c2 sub on Pool
                for half in range(2):
                    hs = slice(half * KH, (half + 1) * KH)
                    ps = psum.tile([P, KH], F32, tag="ps", name=f"ps_s{half}")
                    mm_round("s", half, ps)
                    nc.scalar.copy(out=ps16[:, hs], in_=ps[:])
                    nc.gpsimd.tensor_tensor(
                        out=mnl_s[:, hs], in0=c2h[:P, hs], in1=ps16[:, hs],
                        op=Alu.subtract,
                    )
                # teacher rounds: c2 sub straight from PSUM on DVE (fp32 c2)
                for half in range(2):
                    hs = slice(half * KH, (half + 1) * KH)
                    ps = psum.tile([P, KH], F32, tag="ps", name=f"ps_t{half}")
                    mm_round("t", half, ps)
                    nc.vector.tensor_tensor(
                        out=mnl_t[:, hs], in0=c2b[:P, hs], in1=ps[:],
                        op=Alu.subtract,
                    )

                # exact row minima (= -max logits / min-distance pieces)
                nc.vector.tensor_reduce(
                    out=nm_s_all[:, it : it + 1], in_=mnl_s[:], axis=AxX, op=Alu.min
                )
                nc.vector.tensor_reduce(
                    out=nm_t_all[:, it : it + 1], in_=mnl_t[:], axis=AxX, op=Alu.min
                )

                # e = exp(l - max) = exp(-mnl + nm); accum -> se
                es = scratchpool.tile([P, K], F16, tag="e_s", name="e_s")
                nc.scalar.activation(
                    out=es[:], in_=mnl_s[:], func=Act.Exp,
                    scale=-1.0, bias=nm_s_all[:, it : it + 1],
                    accum_out=se_s_all[:, it : it + 1],
                )
                nc.scalar.activation(
                    out=e_t[:], in_=mnl_t[:], func=Act.Exp,
                    scale=-1.0, bias=nm_t_all[:, it : it + 1],
                    accum_out=se_t_all[:, it : it + 1],
                )

                # dot pieces: d1 = sum(e_t*mnl_s), d2 = sum(e_t*mnl_t)
                prod = scratchpool.tile([P, K], F16, tag="prod", name="prod")
                nc.vector.scalar_tensor_tensor(
                    out=prod[:], in0=mnl_s[:], scalar=1.0, in1=e_t[:],
                    op0=Alu.mult, op1=Alu.mult,
                    accum_out=d1_all[:, it : it + 1],
                )
                prod2 = scratchpool.tile([P, K], F16, tag="prod", name="prod2")
                nc.vector.scalar_tensor_tensor(
                    out=prod2[:], in0=mnl_t[:], scalar=1.0, in1=e_t[:],
                    op0=Alu.mult, op1=Alu.mult,
                    accum_out=d2_all[:, it : it + 1],
                )

            # ---- final combine over (P, NT) ----
            def ftile(name):
                return stats.tile([P, NT], F32, tag=name, name=name)

            dot_all = ftile("dot")
            nc.vector.tensor_sub(out=dot_all[:], in0=d1_all[:], in1=d2_all[:])
            recip_t = ftile("recip_t")
            nc.vector.reciprocal(out=recip_t[:], in_=se_t_all[:])

            # full-range ln via exponent/mantissa split (ACT Ln table only
            # covers a narrow input range): ln(x) = Ln(m) + (e-127)*ln2
            LN2 = 0.6931471805599453
            I32 = mybir.dt.int32

            def full_ln(dst, src, pfx):
                ei = stats.tile([P, NT], I32, tag=pfx + "_ei", name=pfx + "_ei")
                nc.vector.tensor_scalar(
                    out=ei[:], in0=src[:].bitcast(I32), scalar1=23, scalar2=None,
                    op0=Alu.logical_shift_right,
                )
                ef = stats.tile([P, NT], F32, tag=pfx + "_ef", name=pfx + "_ef")
                nc.vector.tensor_copy(out=ef[:], in_=ei[:])
                mi = stats.tile([P, NT], I32, tag=pfx + "_mi", name=pfx + "_mi")
                nc.vector.tensor_scalar(
                    out=mi[:], in0=src[:].bitcast(I32),
                    scalar1=0x007FFFFF, scalar2=0x3F800000,
                    op0=Alu.bitwise_and, op1=Alu.bitwise_or,
                )
                nc.scalar.activation(out=dst[:], in_=mi[:].bitcast(F32), func=Act.Ln)
                ef2 = stats.tile([P, NT], F32, tag=pfx + "_ef2", name=pfx + "_ef2")
                nc.vector.tensor_scalar(
                    out=ef2[:], in0=ef[:], scalar1=LN2, scalar2=127.0 * LN2,
                    op0=Alu.mult, op1=Alu.subtract,
                )
                nc.vector.tensor_add(out=dst[:], in0=dst[:], in1=ef2[:])

            ln_ses = ftile("ln_ses")
            full_ln(ln_ses, se_s_all, "ls")
            ln_set = ftile("ln_set")
            full_ln(ln_set, se_t_all, "lt")
            kl = ftile("kl")
            nc.vector.tensor_mul(out=kl[:], in0=dot_all[:], in1=recip_t[:])
            nc.vector.tensor_add(out=kl[:], in0=kl[:], in1=nm_t_all[:])
            nc.vector.tensor_sub(out=kl[:], in0=kl[:], in1=nm_s_all[:])
            nc.vector.tensor_add(out=kl[:], in0=kl[:], in1=ln_ses[:])
            nc.vector.tensor_sub(out=kl[:], in0=kl[:], in1=ln_set[:])
            nc.vector.tensor_mul(out=kl[:], in0=kl[:], in1=maskT[:])

            packed = stats.tile([P, 4], F32, tag="packed", name="packed")
            fm = ftile("fm")
            nc.vector.tensor_mul(out=fm[:], in0=rsq_pos_all[:], in1=maskT[:])
            nc.vector.reduce_sum(out=packed[:, 0:1], in_=fm[:], axis=AxX)

            posd = ftile("posd")
            nc.scalar.activation(out=posd[:], in_=rsq_pos_all[:], func=Act.Sqrt)
            negd = ftile("negd")
            nc.scalar.activation(out=negd[:], in_=rsq_neg_all[:], func=Act.Sqrt)
            trip = ftile("trip")
            nc.vector.tensor_sub(out=trip[:], in0=posd[:], in1=negd[:])
            nc.scalar.activation(out=trip[:], in_=trip[:], func=Act.Relu, bias=b_margin[:])
            nc.vector.tensor_mul(out=trip[:], in0=trip[:], in1=maskT[:])
            nc.vector.reduce_sum(out=packed[:, 1:2], in_=trip[:], axis=AxX)

            nc.vector.reduce_sum(out=packed[:, 2:3], in_=kl[:], axis=AxX)

            vq = ftile("vq")
            nc.vector.tensor_add(out=vq[:], in0=x2_all[:], in1=nm_s_all[:])
            nc.vector.reduce_sum(out=packed[:, 3:4], in_=vq[:], axis=AxX)

            # partition reduce via fp32 matmul with ones
            pfin = psum.tile([1, 4], F32, tag="ps", name="pfin")
            nc.tensor.matmul(out=pfin[:], lhsT=ones[:], rhs=packed[:], start=True, stop=True)
            out_sb = stats.tile([1, 4], F32, tag="out_sb", name="out_sb")
            nc.scalar.copy(out=out_sb[:], in_=pfin[:])
            nc.sync.dma_start(out=out_d[:], in_=out_sb[:])

            if dump:
                arrs = [nm_t_all, nm_s_all, se_t_all, se_s_all, dot_all,
                        rsq_pos_all, rsq_neg_all, x2_all,
                        recip_t, ln_ses, ln_set, kl]
                for ai, arr in enumerate(arrs):
                    nc.sync.dma_start(
                        out=dbg_d[:, ai * NT : (ai + 1) * NT], in_=arr[:]
                    )

    _split_sync_waits(nc)
    return nc


_NC_CACHE = {}


def _get_nc(dump=False):
    key = "dump" if dump else "nc"
    if key not in _NC_CACHE:
        _NC_CACHE[key] = _build(dump=dump)
    return _NC_CACHE[key]


def kernel(student_features, teacher_features, teacher_codes, codebook, lengths,
           _debug=False, _trace=False, _dump=False):
    S = np.asarray(student_features, dtype=np.float32)
    Tt = np.asarray(teacher_features, dtype=np.float32)
    C = np.asarray(codebook, dtype=np.float32)
    lengths = np.asarray(lengths)

    valid = np.minimum(lengths.astype(np.int64) // STRIDE, T)
    mask = (np.arange(T)[None, :] < valid[:, None]).astype(np.float32)  # (B,T)
    msum = float(mask.sum(dtype=np.float64))

    ctT16 = np.ascontiguousarray(C.T.astype(np.float16))  # (D, K)
    c2 = (C.astype(np.float64) ** 2).sum(1).astype(np.float32)
    c2b = np.ascontiguousarray(np.broadcast_to(c2[None, :], (128, K)))
    c2h = np.ascontiguousarray(np.broadcast_to(c2.astype(np.float16)[None, :], (128, K)))
    S16 = S.astype(np.float16)
    T16 = Tt.astype(np.float16)

    in_maps = []
    for b in range(B):
        in_maps.append(
            {
                "s_nat": np.ascontiguousarray(S16[b]),
                "t_nat": np.ascontiguousarray(T16[b]),
                "tp_nat": np.ascontiguousarray(T16[(b - 1) % B]),
                "st2_tr": np.ascontiguousarray((2.0 * S[b].T).astype(np.float16)),
                "tt2_tr": np.ascontiguousarray((2.0 * Tt[b].T).astype(np.float16)),
                "ct_tr": ctT16,
                "c2b": c2b,
                "c2h": c2h,
                "maskT": np.ascontiguousarray(mask[b].reshape(NT, P).T),
            }
        )

    nc = _get_nc(dump=_dump)
    res = run_bass_kernel_spmd(nc, in_maps, core_ids=list(range(NC)), trace=_trace)
    parts = np.stack([res.results[b]["partials"][0] for b in range(B)])  # (B,4)
    if _dump:
        dbg = np.stack([res.results[b]["dbg"] for b in range(B)])
        return parts, dbg

    F_sum, TR_sum, KL_sum, Q_sum = parts.astype(np.float64).sum(0)
    total = (
        F_sum / D / msum
        + TR_sum / msum
        + KL_sum / msum
        + 0.2 * Q_sum / (B * T * D)
    )
    out = np.array(total, dtype=np.float32)
    if _debug and _trace:
        return out, parts, res.exec_time_ns
    if _debug:
        return out, parts
    return out


# revision 8
# speedup vs baseline: 1.3037x; 1.3037x over previous
"""Trainium2 Bass kernel for nn_CombinedLossExp71 (combined distillation loss).

Sharding: data-parallel over B across 8 cores, codebook replicated.
Each core b handles batch row b (1500 tokens x 512 dims):
  - fp16 matmuls (full PE rate) produce raw logit pieces ps = 2*x.c in PSUM
    per (125-token, 2048-code) half; 4 rounds per tile (s0,s1,t0,t1).
  - teacher path: DVE drains PSUM as mnl_t = c2(fp32) - ps (fp16 out), then
    an exact full-row fp16 min-reduce gives nm_t = -max(logits). No
    subsampling -> exp can never overflow.
  - student path: ACT copies PSUM to SBUF fp16; Pool does the c2 sub
    (all-fp16), DVE min-reduce gives nm_s (= exact min distance piece for
    the VQ term).
  - ACT exp(-mnl + nm) with accum_out yields e_t / se_t / se_s in one op
    per feature.
  - dot = sum(e_t*(l_t - l_s)) is split as  sum(e_t*mnl_s) - sum(e_t*mnl_t):
    two DVE scalar_tensor_tensor passes with fused sum accumulation; the
    difference is taken on (125,12) stats in the final combine. This removes
    the full-K delta pass entirely.
  - feature/triplet/x2 via Pool subs + DVE stt square-accum on fp16 naturals.
  Per-core outputs are 4 partial sums [feature, triplet, kl, vq]; the final
  masked-mean combination happens on host (scalar work only).

Self-contained: hardcodes shapes for B=8, T=1500, D=512, K=4096, STRIDE=320.
"""
import numpy as np

try:
    import concourse.bass as bass
except ImportError:  # environment fallback
    import sys

    sys.path.insert(0, "/opt/trn_rl_repo")
    import concourse.bass as bass

import concourse.tile as tile
from concourse import mybir
from concourse.bass_utils import run_bass_kernel_spmd

B, T, D, K = 8, 1500, 512, 4096
STRIDE = 320
P = 125          # tokens per tile (partition dim)
NT = T // P      # 12 tiles
KH = K // 2      # codes per PSUM round
NC = 8           # cores
F32 = mybir.dt.float32
F16 = mybir.dt.float16
HFD = 2.0 ** -10     # finite-difference step for the d2 = sum(e_t*mnl_t) trick

Act = mybir.ActivationFunctionType
Alu = mybir.AluOpType
AxX = mybir.AxisListType.X


def _split_sync_waits(nc, max_waits=1):
    """This container's walrus supports only one embedded sync-wait per
    instruction; move excess waits onto inserted same-engine NoOps."""
    counter = [0]
    for f in nc.m.functions:
        for bb in f.blocks:
            insts = bb.instructions
            out = []
            changed = False
            for ins in insts:
                si = ins.sync_info
                waits = list(si.on_wait) if si is not None and si.on_wait else []
                if len(waits) > max_waits:
                    changed = True
                    extra, keep = waits[:-max_waits], waits[-max_waits:]
                    for j in range(0, len(extra), max_waits):
                        counter[0] += 1
                        nop = mybir.InstNoOp(
                            name=f"wsplit-{counter[0]}",
                            ins=[],
                            outs=[],
                            engine=ins.engine,
                        )
                        nop.sync_info = mybir.SyncInfo(
                            on_wait=extra[j : j + max_waits], on_update=[]
                        )
                        nc.register_instruction(nop, overwrite=True)
                        out.append(nop)
                    si.on_wait = keep
                out.append(ins)
            if changed:
                insts.clear()
                insts.extend(out)


def _build(dump=False):
    nc = bass.Bass()

    s_nat = nc.dram_tensor("s_nat", [T, D], F16, kind="ExternalInput")
    t_nat = nc.dram_tensor("t_nat", [T, D], F16, kind="ExternalInput")
    tp_nat = nc.dram_tensor("tp_nat", [T, D], F16, kind="ExternalInput")
    st2_tr = nc.dram_tensor("st2_tr", [D, T], F16, kind="ExternalInput")  # 2*S^T
    tt2_tr = nc.dram_tensor("tt2_tr", [D, T], F16, kind="ExternalInput")  # 2*T^T
    ct_tr = nc.dram_tensor("ct_tr", [D, K], F16, kind="ExternalInput")
    c2b_in = nc.dram_tensor("c2b", [128, K], F32, kind="ExternalInput")
    c2h_in = nc.dram_tensor("c2h", [128, K], F16, kind="ExternalInput")
    mask_in = nc.dram_tensor("maskT", [P, NT], F32, kind="ExternalInput")
    out_d = nc.dram_tensor("partials", [1, 4], F32, kind="ExternalOutput")
    dbg_d = nc.dram_tensor("dbg", [P, 12 * NT], F32, kind="ExternalOutput") if dump else None

    with tile.TileContext(nc) as tc:
        with (
            tc.tile_pool(name="const", bufs=1) as const,
            tc.tile_pool(name="stats", bufs=1) as stats,
            tc.tile_pool(name="w", bufs=16) as wpool,
            tc.tile_pool(name="nat", bufs=2) as natpool,
            tc.tile_pool(name="mnl", bufs=2) as mnlpool,
            tc.tile_pool(name="psc", bufs=2) as pscpool,
            tc.tile_pool(name="eh", bufs=2) as ehpool,
            tc.tile_pool(name="scratch", bufs=1) as scratchpool,
            tc.tile_pool(name="sq", bufs=2) as sqpool,
            tc.tile_pool(name="psum", bufs=2, space="PSUM") as psum,
        ):
            # ---- constants ----
            ct_sb = []
            for d in range(4):
                c = const.tile([128, K], F16, tag=f"ct{d}", name=f"ct{d}")
                nc.sync.dma_start(out=c[:], in_=ct_tr[d * 128 : (d + 1) * 128, :])
                ct_sb.append(c)
            c2b = const.tile([128, K], F32, tag="c2b", name="c2b")
            nc.sync.dma_start(out=c2b[:], in_=c2b_in[:])
            c2h = const.tile([128, K], F16, tag="c2h", name="c2h")
            nc.sync.dma_start(out=c2h[:], in_=c2h_in[:])
            maskT = const.tile([P, NT], F32, tag="mask", name="maskT")
            nc.sync.dma_start(out=maskT[:], in_=mask_in[:])
            ones = const.tile([P, 1], F32, tag="ones", name="ones")
            nc.vector.memset(ones[:], 1.0)
            b_margin = const.tile([P, 1], F32, tag="b_margin", name="b_margin")
            nc.vector.memset(b_margin[:], 0.2)

            # ---- per-token stat arrays (col it per tile) ----
            def stat(name):
                return stats.tile([P, NT], F32, tag=name, name=name)

            nm_t_all = stat("nm_t")      # -max(l_t), exact
            nm_s_all = stat("nm_s")      # -max(l_s), exact
            nmh_all = stat("nmh")        # (1+h)*nm_t
            se_t_all = stat("se_t")
            se_s_all = stat("se_s")
            fh_all = stat("fh")          # sum exp((1+h)(nm_t-mnl_t))
            d1_all = stat("d1")          # sum(e_t * mnl_s)
            rsq_pos_all = stat("rsq_pos")
            rsq_neg_all = stat("rsq_neg")
            x2_all = stat("x2")

            trans = {"t": tt2_tr, "s": st2_tr}

            for it in range(NT):
                tok = slice(it * P, (it + 1) * P)

                # natural tiles + feature/triplet/x2 pieces
                s_t = natpool.tile([P, D], F16, tag="s_nat", name="s_t")
                t_t = natpool.tile([P, D], F16, tag="t_nat", name="t_t")
                tp_t = natpool.tile([P, D], F16, tag="tp_nat", name="tp_t")
                nc.sync.dma_start(out=s_t[:], in_=s_nat[tok, :])
                nc.sync.dma_start(out=t_t[:], in_=t_nat[tok, :])
                nc.sync.dma_start(out=tp_t[:], in_=tp_nat[tok, :])

                dpos = sqpool.tile([P, D], F16, tag="dpos", name="dpos")
                dneg = sqpool.tile([P, D], F16, tag="dneg", name="dneg")
                nc.gpsimd.tensor_sub(out=dpos[:], in0=s_t[:], in1=t_t[:])
                nc.gpsimd.tensor_sub(out=dneg[:], in0=s_t[:], in1=tp_t[:])
                sqs = sqpool.tile([P, D], F16, tag="sqscr", name="sqs")
                nc.scalar.activation(
                    out=sqs[:], in_=dpos[:], func=Act.Square,
                    accum_out=rsq_pos_all[:, it : it + 1],
                )
                sqs2 = sqpool.tile([P, D], F16, tag="sqscr", name="sqs2")
                nc.scalar.activation(
                    out=sqs2[:], in_=dneg[:], func=Act.Square,
                    accum_out=rsq_neg_all[:, it : it + 1],
                )
                # x2 on DVE (fused square+sum) to keep ACT under budget
                sqs3 = sqpool.tile([P, D], F16, tag="sqscr", name="sqs3")
                nc.vector.scalar_tensor_tensor(
                    out=sqs3[:], in0=s_t[:], scalar=1.0, in1=s_t[:],
                    op0=Alu.mult, op1=Alu.mult,
                    accum_out=x2_all[:, it : it + 1],
                )

                # stationary weight slices: one DMA per feature, 4 d-chunks
                # side by side in the free dim (3D access pattern)
                w = {}
                for f in ("t", "s"):
                    wt = wpool.tile([128, 4 * P], F16, tag="w", name=f"w_{f}")
                    nc.sync.dma_start(
                        out=wt[:].rearrange("p (c t) -> p c t", c=4),
                        in_=trans[f].rearrange("(c p) t -> p c t", c=4)[:, :, tok],
                    )
                    w[f] = wt

                mnl_t = mnlpool.tile([P, K], F16, tag="mnl_t", name="mnl_t")
                mnl_s = mnlpool.tile([P, K], F16, tag="mnl_s", name="mnl_s")
                ps16 = pscpool.tile([P, K], F16, tag="ps16", name="ps16")
                e_t = ehpool.tile([P, K], F16, tag="e_t", name="e_t")

                def mm_round(f, half, ps):
                    for d in range(4):
                        for kc in range(4):
                            c0 = half * KH + kc * 512
                            nc.tensor.matmul(
                                out=ps[:, kc * 512 : (kc + 1) * 512],
                                lhsT=w[f][:, d * P : (d + 1) * P],
                                rhs=ct_sb[d][:, c0 : c0 + 512],
                                start=(d == 0),
                                stop=(d == 3),
                            )

                # student rounds first: PSUM -> fp16 copy on ACT, c2 sub on Pool
                for half in range(2):
                    hs = slice(half * KH, (half + 1) * KH)
                    ps = psum.tile([P, KH], F32, tag="ps", name=f"ps_s{half}")
                    mm_round("s", half, ps)
                    nc.scalar.copy(out=ps16[:, hs], in_=ps[:])
                # single full-width Pool sub (Pool ops carry ~2us launch cost)
                nc.gpsimd.tensor_tensor(
                    out=mnl_s[:], in0=c2h[:P, :], in1=ps16[:], op=Alu.subtract
                )
                # teacher rounds: c2 sub straight from PSUM on DVE (fp32 c2)
                for half in range(2):
                    hs = slice(half * KH, (half + 1) * KH)
                    ps = psum.tile([P, KH], F32, tag="ps", name=f"ps_t{half}")
                    mm_round("t", half, ps)
                    nc.vector.tensor_tensor(
                        out=mnl_t[:, hs], in0=c2b[:P, hs], in1=ps[:],
                        op=Alu.subtract,
                    )

                # exact row minima (= -max logits / min-distance pieces)
                nc.vector.tensor_reduce(
                    out=nm_s_all[:, it : it + 1], in_=mnl_s[:], axis=AxX, op=Alu.min
                )
                nc.vector.tensor_reduce(
                    out=nm_t_all[:, it : it + 1], in_=mnl_t[:], axis=AxX, op=Alu.min
                )
                nc.vector.tensor_scalar(
                    out=nmh_all[:, it : it + 1], in0=nm_t_all[:, it : it + 1],
                    scalar1=1.0 + HFD, scalar2=None, op0=Alu.mult,
                )

                # e = exp(l - max) = exp(-mnl + nm); accum -> se
                es = scratchpool.tile([P, K], F16, tag="e_s", name="e_s")
                nc.scalar.activation(
                    out=es[:], in_=mnl_s[:], func=Act.Exp,
                    scale=-1.0, bias=nm_s_all[:, it : it + 1],
                    accum_out=se_s_all[:, it : it + 1],
                )
                nc.scalar.activation(
                    out=e_t[:], in_=mnl_t[:], func=Act.Exp,
                    scale=-1.0, bias=nm_t_all[:, it : it + 1],
                    accum_out=se_t_all[:, it : it + 1],
                )
                # perturbed exp: fh = sum exp((1+h)(nm_t - mnl_t)); the final
                # combine turns this into d2 = sum(e_t*mnl_t) by finite diff
                es2 = scratchpool.tile([P, K], F16, tag="e_s", name="e_h")
                nc.scalar.activation(
                    out=es2[:], in_=mnl_t[:], func=Act.Exp,
                    scale=-(1.0 + HFD), bias=nmh_all[:, it : it + 1],
                    accum_out=fh_all[:, it : it + 1],
                )

                # d1 = sum(e_t*mnl_s) (fused multiply+sum on DVE)
                prod = scratchpool.tile([P, K], F16, tag="prod", name="prod")
                nc.vector.scalar_tensor_tensor(
                    out=prod[:], in0=mnl_s[:], scalar=1.0, in1=e_t[:],
                    op0=Alu.mult, op1=Alu.mult,
                    accum_out=d1_all[:, it : it + 1],
                )

            # ---- final combine over (P, NT) ----
            def ftile(name):
                return stats.tile([P, NT], F32, tag=name, name=name)

            # d2 = nm_t*se_t - (fh - se_t)/h, then dot = d1 - d2
            d2_all = ftile("d2")
            nc.vector.tensor_sub(out=d2_all[:], in0=fh_all[:], in1=se_t_all[:])
            nc.vector.tensor_scalar(
                out=d2_all[:], in0=d2_all[:], scalar1=1.0 / HFD, scalar2=None,
                op0=Alu.mult,
            )
            nmse = ftile("nmse")
            nc.vector.tensor_mul(out=nmse[:], in0=nm_t_all[:], in1=se_t_all[:])
            nc.vector.tensor_sub(out=d2_all[:], in0=nmse[:], in1=d2_all[:])
            dot_all = ftile("dot")
            nc.vector.tensor_sub(out=dot_all[:], in0=d1_all[:], in1=d2_all[:])
            recip_t = ftile("recip_t")
            nc.vector.reciprocal(out=recip_t[:], in_=se_t_all[:])

            # full-range ln via exponent/mantissa split (ACT Ln table only
            # covers a narrow input range): ln(x) = Ln(m) + (e-127)*ln2
            LN2 = 0.6931471805599453
            I32 = mybir.dt.int32

            def full_ln(dst, src, pfx):
                ei = stats.tile([P, NT], I32, tag=pfx + "_ei", name=pfx + "_ei")
                nc.vector.tensor_scalar(
                    out=ei[:], in0=src[:].bitcast(I32), scalar1=23, scalar2=None,
                    op0=Alu.logical_shift_right,
                )
                ef = stats.tile([P, NT], F32, tag=pfx + "_ef", name=pfx + "_ef")
                nc.vector.tensor_copy(out=ef[:], in_=ei[:])
                mi = stats.tile([P, NT], I32, tag=pfx + "_mi", name=pfx + "_mi")
                nc.vector.tensor_scalar(
                    out=mi[:], in0=src[:].bitcast(I32),
                    scalar1=0x007FFFFF, scalar2=0x3F800000,
                    op0=Alu.bitwise_and, op1=Alu.bitwise_or,
                )
                nc.scalar.activation(out=dst[:], in_=mi[:].bitcast(F32), func=Act.Ln)
                ef2 = stats.tile([P, NT], F32, tag=pfx + "_ef2", name=pfx + "_ef2")
                nc.vector.tensor_scalar(
                    out=ef2[:], in0=ef[:], scalar1=LN2, scalar2=127.0 * LN2,
                    op0=Alu.mult, op1=Alu.subtract,
                )
                nc.vector.tensor_add(out=dst[:], in0=dst[:], in1=ef2[:])

            # ln(se_s/se_t) in one full-range ln of the ratio
            ratio = ftile("ratio")
            nc.vector.tensor_mul(out=ratio[:], in0=se_s_all[:], in1=recip_t[:])
            ln_ratio = ftile("ln_ratio")
            full_ln(ln_ratio, ratio, "lr")
            kl = ftile("kl")
            nc.vector.tensor_mul(out=kl[:], in0=dot_all[:], in1=recip_t[:])
            nc.vector.tensor_add(out=kl[:], in0=kl[:], in1=nm_t_all[:])
            nc.vector.tensor_sub(out=kl[:], in0=kl[:], in1=nm_s_all[:])
            nc.vector.tensor_add(out=kl[:], in0=kl[:], in1=ln_ratio[:])
            nc.vector.tensor_mul(out=kl[:], in0=kl[:], in1=maskT[:])

            packed = stats.tile([P, 4], F32, tag="packed", name="packed")
            fm = ftile("fm")
            nc.vector.tensor_mul(out=fm[:], in0=rsq_pos_all[:], in1=maskT[:])
            nc.vector.reduce_sum(out=packed[:, 0:1], in_=fm[:], axis=AxX)

            posd = ftile("posd")
            nc.scalar.activation(out=posd[:], in_=rsq_pos_all[:], func=Act.Sqrt)
            negd = ftile("negd")
            nc.scalar.activation(out=negd[:], in_=rsq_neg_all[:], func=Act.Sqrt)
            trip = ftile("trip")
            nc.vector.tensor_sub(out=trip[:], in0=posd[:], in1=negd[:])
            nc.scalar.activation(out=trip[:], in_=trip[:], func=Act.Relu, bias=b_margin[:])
            nc.vector.tensor_mul(out=trip[:], in0=trip[:], in1=maskT[:])
            nc.vector.reduce_sum(out=packed[:, 1:2], in_=trip[:], axis=AxX)

            nc.vector.reduce_sum(out=packed[:, 2:3], in_=kl[:], axis=AxX)

            vq = ftile("vq")
            nc.vector.tensor_add(out=vq[:], in0=x2_all[:], in1=nm_s_all[:])
            nc.vector.reduce_sum(out=packed[:, 3:4], in_=vq[:], axis=AxX)

            # partition reduce via fp32 matmul with ones
            pfin = psum.tile([1, 4], F32, tag="ps", name="pfin")
            nc.tensor.matmul(out=pfin[:], lhsT=ones[:], rhs=packed[:], start=True, stop=True)
            out_sb = stats.tile([1, 4], F32, tag="out_sb", name="out_sb")
            nc.scalar.copy(out=out_sb[:], in_=pfin[:])
            nc.sync.dma_start(out=out_d[:], in_=out_sb[:])

            if dump:
                arrs = [nm_t_all, nm_s_all, se_t_all, se_s_all, dot_all,
                        rsq_pos_all, rsq_neg_all, x2_all,
                        recip_t, ln_ratio, d1_all, kl]
                for ai, arr in enumerate(arrs):
                    nc.sync.dma_start(
                        out=dbg_d[:, ai * NT : (ai + 1) * NT], in_=arr[:]
                    )

    _split_sync_waits(nc)
    return nc


_NC_CACHE = {}


def _get_nc(dump=False):
    key = "dump" if dump else "nc"
    if key not in _NC_CACHE:
        _NC_CACHE[key] = _build(dump=dump)
    return _NC_CACHE[key]


def kernel(student_features, teacher_features, teacher_codes, codebook, lengths,
           _debug=False, _trace=False, _dump=False):
    S = np.asarray(student_features, dtype=np.float32)
    Tt = np.asarray(teacher_features, dtype=np.float32)
    C = np.asarray(codebook, dtype=np.float32)
    lengths = np.asarray(lengths)

    valid = np.minimum(lengths.astype(np.int64) // STRIDE, T)
    mask = (np.arange(T)[None, :] < valid[:, None]).astype(np.float32)  # (B,T)
    msum = float(mask.sum(dtype=np.float64))

    ctT16 = np.ascontiguousarray(C.T.astype(np.float16))  # (D, K)
    c2 = (C.astype(np.float64) ** 2).sum(1).astype(np.float32)
    c2b = np.ascontiguousarray(np.broadcast_to(c2[None, :], (128, K)))
    c2h = np.ascontiguousarray(np.broadcast_to(c2.astype(np.float16)[None, :], (128, K)))
    S16 = S.astype(np.float16)
    T16 = Tt.astype(np.float16)

    in_maps = []
    for b in range(B):
        in_maps.append(
            {
                "s_nat": np.ascontiguousarray(S16[b]),
                "t_nat": np.ascontiguousarray(T16[b]),
                "tp_nat": np.ascontiguousarray(T16[(b - 1) % B]),
                "st2_tr": np.ascontiguousarray((2.0 * S[b].T).astype(np.float16)),
                "tt2_tr": np.ascontiguousarray((2.0 * Tt[b].T).astype(np.float16)),
                "ct_tr": ctT16,
                "c2b": c2b,
                "c2h": c2h,
                "maskT": np.ascontiguousarray(mask[b].reshape(NT, P).T),
            }
        )

    nc = _get_nc(dump=_dump)
    res = run_bass_kernel_spmd(nc, in_maps, core_ids=list(range(NC)), trace=_trace)
    parts = np.stack([res.results[b]["partials"][0] for b in range(B)])  # (B,4)
    if _dump:
        dbg = np.stack([res.results[b]["dbg"] for b in range(B)])
        return parts, dbg

    F_sum, TR_sum, KL_sum, Q_sum = parts.astype(np.float64).sum(0)
    total = (
        F_sum / D / msum
        + TR_sum / msum
        + KL_sum / msum
        + 0.2 * Q_sum / (B * T * D)
    )
    out = np.array(total, dtype=np.float32)
    if _debug and _trace:
        return out, parts, res.exec_time_ns
    if _debug:
        return out, parts
    return out


# revision 9
# speedup vs baseline: 1.3229x; 1.0148x over previous
"""Trainium2 Bass kernel for nn_CombinedLossExp71 (combined distillation loss).

Sharding: data-parallel over B across 8 cores, codebook replicated.
Each core b handles batch row b (1500 tokens x 512 dims):
  - fp16 matmuls (full PE rate) produce raw logit pieces ps = 2*x.c in PSUM
    per (125-token, 2048-code) half; 4 rounds per tile (s0,s1,t0,t1).
  - teacher path: DVE drains PSUM as mnl_t = c2(fp32) - ps (fp16 out), then
    an exact full-row fp16 min-reduce gives nm_t = -max(logits). No
    subsampling -> exp can never overflow.
  - student path: ACT copies PSUM to SBUF fp16; Pool does the c2 sub
    (all-fp16), DVE min-reduce gives nm_s (= exact min distance piece for
    the VQ term).
  - ACT exp(-mnl + nm) with accum_out yields e_t / se_t / se_s in one op
    per feature.
  - dot = sum(e_t*(l_t - l_s)) is split as  sum(e_t*mnl_s) - sum(e_t*mnl_t):
    two DVE scalar_tensor_tensor passes with fused sum accumulation; the
    difference is taken on (125,12) stats in the final combine. This removes
    the full-K delta pass entirely.
  - feature/triplet/x2 via Pool subs + DVE stt square-accum on fp16 naturals.
  Per-core outputs are 4 partial sums [feature, triplet, kl, vq]; the final
  masked-mean combination happens on host (scalar work only).

Self-contained: hardcodes shapes for B=8, T=1500, D=512, K=4096, STRIDE=320.
"""
import numpy as np

try:
    import concourse.bass as bass
except ImportError:  # environment fallback
    import sys

    sys.path.insert(0, "/opt/trn_rl_repo")
    import concourse.bass as bass

import concourse.tile as tile
from concourse import mybir
from concourse.bass_utils import run_bass_kernel_spmd

B, T, D, K = 8, 1500, 512, 4096
STRIDE = 320
P = 125          # tokens per tile (partition dim)
NT = T // P      # 12 tiles
KH = K // 2      # codes per PSUM round
NC = 8           # cores
F32 = mybir.dt.float32
F16 = mybir.dt.float16
HFD = 2.0 ** -10     # finite-difference step for the d2 = sum(e_t*mnl_t) trick

Act = mybir.ActivationFunctionType
Alu = mybir.AluOpType
AxX = mybir.AxisListType.X


def _split_sync_waits(nc, max_waits=1):
    """This container's walrus supports only one embedded sync-wait per
    instruction; move excess waits onto inserted same-engine NoOps."""
    counter = [0]
    for f in nc.m.functions:
        for bb in f.blocks:
            insts = bb.instructions
            out = []
            changed = False
            for ins in insts:
                si = ins.sync_info
                waits = list(si.on_wait) if si is not None and si.on_wait else []
                if len(waits) > max_waits:
                    changed = True
                    extra, keep = waits[:-max_waits], waits[-max_waits:]
                    for j in range(0, len(extra), max_waits):
                        counter[0] += 1
                        nop = mybir.InstNoOp(
                            name=f"wsplit-{counter[0]}",
                            ins=[],
                            outs=[],
                            engine=ins.engine,
                        )
                        nop.sync_info = mybir.SyncInfo(
                            on_wait=extra[j : j + max_waits], on_update=[]
                        )
                        nc.register_instruction(nop, overwrite=True)
                        out.append(nop)
                    si.on_wait = keep
                out.append(ins)
            if changed:
                insts.clear()
                insts.extend(out)


def _build(dump=False):
    nc = bass.Bass()

    s_nat = nc.dram_tensor("s_nat", [T, D], F16, kind="ExternalInput")
    t_nat = nc.dram_tensor("t_nat", [T, D], F16, kind="ExternalInput")
    tp_nat = nc.dram_tensor("tp_nat", [T, D], F16, kind="ExternalInput")
    st2_tr = nc.dram_tensor("st2_tr", [D, T], F16, kind="ExternalInput")  # 2*S^T
    tt2_tr = nc.dram_tensor("tt2_tr", [D, T], F16, kind="ExternalInput")  # 2*T^T
    ct_tr = nc.dram_tensor("ct_tr", [D, K], F16, kind="ExternalInput")
    c2h_in = nc.dram_tensor("c2h", [128, K], F16, kind="ExternalInput")
    mask_in = nc.dram_tensor("maskT", [P, NT], F32, kind="ExternalInput")
    out_d = nc.dram_tensor("partials", [1, 4], F32, kind="ExternalOutput")
    dbg_d = nc.dram_tensor("dbg", [P, 12 * NT], F32, kind="ExternalOutput") if dump else None

    with tile.TileContext(nc) as tc:
        with (
            tc.tile_pool(name="const", bufs=1) as const,
            tc.tile_pool(name="stats", bufs=1) as stats,
            tc.tile_pool(name="w", bufs=16) as wpool,
            tc.tile_pool(name="nat", bufs=2) as natpool,
            tc.tile_pool(name="mnl", bufs=2) as mnlpool,
            tc.tile_pool(name="psc", bufs=2) as pscpool,
            tc.tile_pool(name="eh", bufs=2) as ehpool,
            tc.tile_pool(name="scratch", bufs=1) as scratchpool,
            tc.tile_pool(name="sq", bufs=2) as sqpool,
            tc.tile_pool(name="psum", bufs=2, space="PSUM") as psum,
        ):
            # ---- constants ----
            ct_sb = []
            for d in range(4):
                c = const.tile([128, K], F16, tag=f"ct{d}", name=f"ct{d}")
                nc.sync.dma_start(out=c[:], in_=ct_tr[d * 128 : (d + 1) * 128, :])
                ct_sb.append(c)
            c2h = const.tile([128, K], F16, tag="c2h", name="c2h")
            nc.sync.dma_start(out=c2h[:], in_=c2h_in[:])
            maskT = const.tile([P, NT], F32, tag="mask", name="maskT")
            nc.sync.dma_start(out=maskT[:], in_=mask_in[:])
            ones = const.tile([P, 1], F32, tag="ones", name="ones")
            nc.vector.memset(ones[:], 1.0)
            b_margin = const.tile([P, 1], F32, tag="b_margin", name="b_margin")
            nc.vector.memset(b_margin[:], 0.2)

            # ---- per-token stat arrays (col it per tile) ----
            def stat(name):
                return stats.tile([P, NT], F32, tag=name, name=name)

            nm_t_all = stats.tile([P, NT], F16, tag="nm_t", name="nm_t")  # -max(l_t)
            nm_s_all = stats.tile([P, NT], F16, tag="nm_s", name="nm_s")  # -max(l_s)
            se_t_all = stat("se_t")
            se_s_all = stat("se_s")
            fh_all = stat("fh")          # sum exp((1+h)(nm_t-mnl_t))
            d1_all = stat("d1")          # sum(e_t * mnl_s)
            rsq_pos_all = stat("rsq_pos")
            rsq_neg_all = stat("rsq_neg")
            x2_all = stat("x2")

            trans = {"t": tt2_tr, "s": st2_tr}

            for it in range(NT):
                tok = slice(it * P, (it + 1) * P)

                # natural tiles + feature/triplet/x2 pieces
                s_t = natpool.tile([P, D], F16, tag="s_nat", name="s_t")
                t_t = natpool.tile([P, D], F16, tag="t_nat", name="t_t")
                tp_t = natpool.tile([P, D], F16, tag="tp_nat", name="tp_t")
                nc.sync.dma_start(out=s_t[:], in_=s_nat[tok, :])
                nc.sync.dma_start(out=t_t[:], in_=t_nat[tok, :])
                nc.sync.dma_start(out=tp_t[:], in_=tp_nat[tok, :])

                dpos = sqpool.tile([P, D], F16, tag="dpos", name="dpos")
                dneg = sqpool.tile([P, D], F16, tag="dneg", name="dneg")
                nc.gpsimd.tensor_sub(out=dpos[:], in0=s_t[:], in1=t_t[:])
                nc.gpsimd.tensor_sub(out=dneg[:], in0=s_t[:], in1=tp_t[:])
                sqs = sqpool.tile([P, D], F16, tag="sqscr", name="sqs")
                nc.scalar.activation(
                    out=sqs[:], in_=dpos[:], func=Act.Square,
                    accum_out=rsq_pos_all[:, it : it + 1],
                )
                sqs2 = sqpool.tile([P, D], F16, tag="sqscr", name="sqs2")
                nc.scalar.activation(
                    out=sqs2[:], in_=dneg[:], func=Act.Square,
                    accum_out=rsq_neg_all[:, it : it + 1],
                )
                # x2 on DVE (fused square+sum) to keep ACT under budget
                sqs3 = sqpool.tile([P, D], F16, tag="sqscr", name="sqs3")
                nc.vector.scalar_tensor_tensor(
                    out=sqs3[:], in0=s_t[:], scalar=1.0, in1=s_t[:],
                    op0=Alu.mult, op1=Alu.mult,
                    accum_out=x2_all[:, it : it + 1],
                )

                # stationary weight slices: one DMA per feature, 4 d-chunks
                # side by side in the free dim (3D access pattern)
                w = {}
                for f in ("t", "s"):
                    wt = wpool.tile([128, 4 * P], F16, tag="w", name=f"w_{f}")
                    nc.sync.dma_start(
                        out=wt[:].rearrange("p (c t) -> p c t", c=4),
                        in_=trans[f].rearrange("(c p) t -> p c t", c=4)[:, :, tok],
                    )
                    w[f] = wt

                mnl_t = mnlpool.tile([P, K], F16, tag="mnl_t", name="mnl_t")
                mnl_s = mnlpool.tile([P, K], F16, tag="mnl_s", name="mnl_s")
                ps16 = pscpool.tile([P, K], F16, tag="ps16", name="ps16")
                e_t = ehpool.tile([P, K], F16, tag="e_t", name="e_t")

                def mm_round(f, half, ps):
                    for d in range(4):
                        for kc in range(4):
                            c0 = half * KH + kc * 512
                            nc.tensor.matmul(
                                out=ps[:, kc * 512 : (kc + 1) * 512],
                                lhsT=w[f][:, d * P : (d + 1) * P],
                                rhs=ct_sb[d][:, c0 : c0 + 512],
                                start=(d == 0),
                                stop=(d == 3),
                            )

                # student rounds first: PSUM -> fp16 copy on ACT, c2 sub on Pool
                for half in range(2):
                    hs = slice(half * KH, (half + 1) * KH)
                    ps = psum.tile([P, KH], F32, tag="ps", name=f"ps_s{half}")
                    mm_round("s", half, ps)
                    nc.scalar.copy(out=ps16[:, hs], in_=ps[:])
                # single full-width Pool sub (Pool ops carry ~2us launch cost)
                nc.gpsimd.tensor_tensor(
                    out=mnl_s[:], in0=c2h[:P, :], in1=ps16[:], op=Alu.subtract
                )
                # teacher rounds: c2 sub straight from PSUM on DVE (fp32 c2)
                for half in range(2):
                    hs = slice(half * KH, (half + 1) * KH)
                    ps = psum.tile([P, KH], F32, tag="ps", name=f"ps_t{half}")
                    mm_round("t", half, ps)
                    nc.vector.tensor_tensor(
                        out=mnl_t[:, hs], in0=c2h[:P, hs], in1=ps[:],
                        op=Alu.subtract,
                    )

                # exact row minima (= -max logits / min-distance pieces)
                nc.vector.tensor_reduce(
                    out=nm_s_all[:, it : it + 1], in_=mnl_s[:], axis=AxX, op=Alu.min
                )
                nc.vector.tensor_reduce(
                    out=nm_t_all[:, it : it + 1], in_=mnl_t[:], axis=AxX, op=Alu.min
                )
                # e = exp(l - max) = exp(-mnl + nm); accum -> se
                es = scratchpool.tile([P, K], F16, tag="e_s", name="e_s")
                nc.scalar.activation(
                    out=es[:], in_=mnl_s[:], func=Act.Exp,
                    scale=-1.0, bias=nm_s_all[:, it : it + 1],
                    accum_out=se_s_all[:, it : it + 1],
                )
                nc.scalar.activation(
                    out=e_t[:], in_=mnl_t[:], func=Act.Exp,
                    scale=-1.0, bias=nm_t_all[:, it : it + 1],
                    accum_out=se_t_all[:, it : it + 1],
                )
                # perturbed exp: fh = sum exp((1+h)(nm_t - mnl_t)); the final
                # combine turns this into d2 = sum(e_t*mnl_t) by finite diff
                es2 = scratchpool.tile([P, K], F16, tag="e_s", name="e_h")
                nc.scalar.activation(
                    out=es2[:], in_=mnl_t[:], func=Act.Exp,
                    scale=-(1.0 + HFD), bias=nm_t_all[:, it : it + 1],
                    accum_out=fh_all[:, it : it + 1],
                )

                # d1 = sum(e_t*mnl_s) (fused multiply+sum on DVE)
                prod = scratchpool.tile([P, K], F16, tag="prod", name="prod")
                nc.vector.scalar_tensor_tensor(
                    out=prod[:], in0=mnl_s[:], scalar=1.0, in1=e_t[:],
                    op0=Alu.mult, op1=Alu.mult,
                    accum_out=d1_all[:, it : it + 1],
                )

            # ---- final combine over (P, NT) ----
            def ftile(name):
                return stats.tile([P, NT], F32, tag=name, name=name)

            # fp32 copies of the fp16 nm stats
            nm_t_f = ftile("nm_t_f")
            nc.vector.tensor_copy(out=nm_t_f[:], in_=nm_t_all[:])
            nm_s_f = ftile("nm_s_f")
            nc.vector.tensor_copy(out=nm_s_f[:], in_=nm_s_all[:])
            # fh used bias nm (not (1+h)nm): correct by exp(h*nm), exact since
            # h*nm is exact in fp32
            hnm = ftile("hnm")
            nc.vector.tensor_scalar(
                out=hnm[:], in0=nm_t_f[:], scalar1=HFD, scalar2=None, op0=Alu.mult,
            )
            corr = ftile("corr")
            nc.scalar.activation(out=corr[:], in_=hnm[:], func=Act.Exp)
            fhc = ftile("fhc")
            nc.vector.tensor_mul(out=fhc[:], in0=fh_all[:], in1=corr[:])
            # d2 = nm_t*se_t - (fhc - se_t)/h, then dot = d1 - d2
            d2_all = ftile("d2")
            nc.vector.tensor_sub(out=d2_all[:], in0=fhc[:], in1=se_t_all[:])
            nc.vector.tensor_scalar(
                out=d2_all[:], in0=d2_all[:], scalar1=1.0 / HFD, scalar2=None,
                op0=Alu.mult,
            )
            nmse = ftile("nmse")
            nc.vector.tensor_mul(out=nmse[:], in0=nm_t_f[:], in1=se_t_all[:])
            nc.vector.tensor_sub(out=d2_all[:], in0=nmse[:], in1=d2_all[:])
            dot_all = ftile("dot")
            nc.vector.tensor_sub(out=dot_all[:], in0=d1_all[:], in1=d2_all[:])
            recip_t = ftile("recip_t")
            nc.vector.reciprocal(out=recip_t[:], in_=se_t_all[:])

            # full-range ln via exponent/mantissa split (ACT Ln table only
            # covers a narrow input range): ln(x) = Ln(m) + (e-127)*ln2
            LN2 = 0.6931471805599453
            I32 = mybir.dt.int32

            def full_ln(dst, src, pfx):
                ei = stats.tile([P, NT], I32, tag=pfx + "_ei", name=pfx + "_ei")
                nc.vector.tensor_scalar(
                    out=ei[:], in0=src[:].bitcast(I32), scalar1=23, scalar2=None,
                    op0=Alu.logical_shift_right,
                )
                ef = stats.tile([P, NT], F32, tag=pfx + "_ef", name=pfx + "_ef")
                nc.vector.tensor_copy(out=ef[:], in_=ei[:])
                mi = stats.tile([P, NT], I32, tag=pfx + "_mi", name=pfx + "_mi")
                nc.vector.tensor_scalar(
                    out=mi[:], in0=src[:].bitcast(I32),
                    scalar1=0x007FFFFF, scalar2=0x3F800000,
                    op0=Alu.bitwise_and, op1=Alu.bitwise_or,
                )
                nc.scalar.activation(out=dst[:], in_=mi[:].bitcast(F32), func=Act.Ln)
                ef2 = stats.tile([P, NT], F32, tag=pfx + "_ef2", name=pfx + "_ef2")
                nc.vector.tensor_scalar(
                    out=ef2[:], in0=ef[:], scalar1=LN2, scalar2=127.0 * LN2,
                    op0=Alu.mult, op1=Alu.subtract,
                )
                nc.vector.tensor_add(out=dst[:], in0=dst[:], in1=ef2[:])

            # ln(se_s/se_t) in one full-range ln of the ratio
            ratio = ftile("ratio")
            nc.vector.tensor_mul(out=ratio[:], in0=se_s_all[:], in1=recip_t[:])
            ln_ratio = ftile("ln_ratio")
            full_ln(ln_ratio, ratio, "lr")
            kl = ftile("kl")
            nc.vector.tensor_mul(out=kl[:], in0=dot_all[:], in1=recip_t[:])
            nc.vector.tensor_add(out=kl[:], in0=kl[:], in1=nm_t_f[:])
            nc.vector.tensor_sub(out=kl[:], in0=kl[:], in1=nm_s_f[:])
            nc.vector.tensor_add(out=kl[:], in0=kl[:], in1=ln_ratio[:])
            nc.vector.tensor_mul(out=kl[:], in0=kl[:], in1=maskT[:])

            packed = stats.tile([P, 4], F32, tag="packed", name="packed")
            fm = ftile("fm")
            nc.vector.tensor_mul(out=fm[:], in0=rsq_pos_all[:], in1=maskT[:])
            nc.vector.reduce_sum(out=packed[:, 0:1], in_=fm[:], axis=AxX)

            posd = ftile("posd")
            nc.scalar.activation(out=posd[:], in_=rsq_pos_all[:], func=Act.Sqrt)
            negd = ftile("negd")
            nc.scalar.activation(out=negd[:], in_=rsq_neg_all[:], func=Act.Sqrt)
            trip = ftile("trip")
            nc.vector.tensor_sub(out=trip[:], in0=posd[:], in1=negd[:])
            nc.scalar.activation(out=trip[:], in_=trip[:], func=Act.Relu, bias=b_margin[:])
            nc.vector.tensor_mul(out=trip[:], in0=trip[:], in1=maskT[:])
            nc.vector.reduce_sum(out=packed[:, 1:2], in_=trip[:], axis=AxX)

            nc.vector.reduce_sum(out=packed[:, 2:3], in_=kl[:], axis=AxX)

            vq = ftile("vq")
            nc.vector.tensor_add(out=vq[:], in0=x2_all[:], in1=nm_s_f[:])
            nc.vector.reduce_sum(out=packed[:, 3:4], in_=vq[:], axis=AxX)

            # partition reduce via fp32 matmul with ones
            pfin = psum.tile([1, 4], F32, tag="ps", name="pfin")
            nc.tensor.matmul(out=pfin[:], lhsT=ones[:], rhs=packed[:], start=True, stop=True)
            out_sb = stats.tile([1, 4], F32, tag="out_sb", name="out_sb")
            nc.scalar.copy(out=out_sb[:], in_=pfin[:])
            nc.sync.dma_start(out=out_d[:], in_=out_sb[:])

            if dump:
                arrs = [nm_t_f, nm_s_f, se_t_all, se_s_all, dot_all,
                        rsq_pos_all, rsq_neg_all, x2_all,
                        recip_t, ln_ratio, d1_all, kl]
                for ai, arr in enumerate(arrs):
                    nc.sync.dma_start(
                        out=dbg_d[:, ai * NT : (ai + 1) * NT], in_=arr[:]
                    )

    _split_sync_waits(nc)
    return nc


_NC_CACHE = {}


def _get_nc(dump=False):
    key = "dump" if dump else "nc"
    if key not in _NC_CACHE:
        _NC_CACHE[key] = _build(dump=dump)
    return _NC_CACHE[key]


def kernel(student_features, teacher_features, teacher_codes, codebook, lengths,
           _debug=False, _trace=False, _dump=False):
    S = np.asarray(student_features, dtype=np.float32)
    Tt = np.asarray(teacher_features, dtype=np.float32)
    C = np.asarray(codebook, dtype=np.float32)
    lengths = np.asarray(lengths)

    valid = np.minimum(lengths.astype(np.int64) // STRIDE, T)
    mask = (np.arange(T)[None, :] < valid[:, None]).astype(np.float32)  # (B,T)
    msum = float(mask.sum(dtype=np.float64))

    ctT16 = np.ascontiguousarray(C.T.astype(np.float16))  # (D, K)
    c2 = (C.astype(np.float64) ** 2).sum(1).astype(np.float32)
    c2h = np.ascontiguousarray(np.broadcast_to(c2.astype(np.float16)[None, :], (128, K)))
    S16 = S.astype(np.float16)
    T16 = Tt.astype(np.float16)

    in_maps = []
    for b in range(B):
        in_maps.append(
            {
                "s_nat": np.ascontiguousarray(S16[b]),
                "t_nat": np.ascontiguousarray(T16[b]),
                "tp_nat": np.ascontiguousarray(T16[(b - 1) % B]),
                "st2_tr": np.ascontiguousarray((2.0 * S[b].T).astype(np.float16)),
                "tt2_tr": np.ascontiguousarray((2.0 * Tt[b].T).astype(np.float16)),
                "ct_tr": ctT16,
                "c2h": c2h,
                "maskT": np.ascontiguousarray(mask[b].reshape(NT, P).T),
            }
        )

    nc = _get_nc(dump=_dump)
    res = run_bass_kernel_spmd(nc, in_maps, core_ids=list(range(NC)), trace=_trace)
    parts = np.stack([res.results[b]["partials"][0] for b in range(B)])  # (B,4)
    if _dump:
        dbg = np.stack([res.results[b]["dbg"] for b in range(B)])
        return parts, dbg

    F_sum, TR_sum, KL_sum, Q_sum = parts.astype(np.float64).sum(0)
    total = (
        F_sum / D / msum
        + TR_sum / msum
        + KL_sum / msum
        + 0.2 * Q_sum / (B * T * D)
    )
    out = np.array(total, dtype=np.float32)
    if _debug and _trace:
        return out, parts, res.exec_time_ns
    if _debug:
        return out, parts
    return out
